# revision 24
# baseline (speedup 1.0000x reference)
# Trainium2 Bass kernel for nn_LAB_42906723287350.
#
#   probs = softmax(choice_parameters, axis=0); s = x @ probs
#   out = mix0*multilinear(sigmoid(lut); s) + mix1*clip(s0+s1+s2-2,0,1)
#         + mix2*(s0+s1+s2>=2)
#
# Data parallel over 8 cores (R rows each).  Per core (v3 design):
#   Row space is partition-blocked: partition p owns rows
#   [p*R/128, (p+1)*R/128).  x loaded in 16 chunks [128, 4096] f32
#   (16 KB/partition DMA lines; chunk n covers rows RP*p + 64n + t).
#   Each chunk: 32 PE transposes [128,128] -> PSUM (4/bank) -> ACT/DVE
#   copy to SBUF (bf16 cast) -> 32 tiny matmuls with the x-transpose as
#   STATIONARY and a constant [128,10] bf16 "pab" as MOVING operand:
#   psum[p, 10k+2v+t] = s_v(row) for v in {s0..s3, s0+s1+s2} -- s lands
#   in natural layout, no back-transposes.  One PSUM bank per chunk,
#   one ACT copy -> SBUF.  Poly (16-leaf Horner + add path) runs per
#   2-chunk batch spread across ACT/DVE/Pool; each batch stores with a
#   single full-rate DMA (512 B runs).
import numpy as np
import ml_dtypes

import concourse.bass as bass
import concourse.mybir as mybir
import concourse.tile as tile
from concourse import bacc
from concourse.bass_utils import run_bass_kernel_spmd

N_CORES = 8
B_FULL = 1048576
CIN = 64
F32 = mybir.dt.float32
F32R = mybir.dt.float32r
BF16 = mybir.dt.bfloat16
TPC = 64          # rows per partition per chunk
CHUNK_ROWS = 128 * TPC          # 8192
SUB_PER_CHUNK = TPC // 2        # 32 transpose sub-tiles per chunk
BATCH_CHUNKS = 2                # poly batch = 2 chunks = 16384 rows
ALU = mybir.AluOpType
AF = mybir.ActivationFunctionType


def build_nc(R, tpc=TPC, batch_chunks=BATCH_CHUNKS, xin_bufs=4, xt_bufs=6,
             sn_bufs=3, tmp_bufs=2, pT_bufs=4, pS_bufs=3, split_first=4,
             tail_single=True, tail_half=False, poly_deprio=250,
             x_f32r=True, tail_halves=False, pe_warmup=0):
    XDT = F32R if x_f32r else F32
    TPCl = tpc
    CHUNK_ROWSl = 128 * TPCl
    SUBl = TPCl // 2
    BCl = batch_chunks
    assert R % (BCl * CHUNK_ROWSl) == 0
    n_chunks = R // CHUNK_ROWSl
    n_batches = n_chunks // BCl
    MB = BCl * TPCl     # out rows per partition per batch

    nc = bacc.Bacc()
    x_d = nc.dram_tensor("x", [R, CIN], XDT, kind="ExternalInput")
    pab_d = nc.dram_tensor("pab", [128, 10], BF16, kind="ExternalInput")
    coef_d = nc.dram_tensor("coef", [128, 18], F32, kind="ExternalInput")
    out_d = nc.dram_tensor("out", [R, 1], F32, kind="ExternalOutput")
    ident_d = nc.dram_tensor("ident", [128, 128], XDT, kind="ExternalInput")

    # x row = (R/128)*p + 64*n + t  (16 KB contiguous per partition line)
    xv = x_d[:].rearrange("(p n t) c -> n p (t c)", p=128, n=n_chunks, t=TPCl)
    # out row = (R/128)*p + m ; one full-rate store per poly batch
    outv = out_d[:].rearrange("(p m) one -> p (m one)", p=128)

    with tile.TileContext(nc) as tc:
        with (
            tc.tile_pool(name="const", bufs=1) as cpool,
            tc.tile_pool(name="xin", bufs=xin_bufs) as xpool,
            tc.tile_pool(name="xtsb", bufs=xt_bufs) as xtpool,
            tc.tile_pool(name="snat", bufs=sn_bufs) as snpool,
            tc.tile_pool(name="tmp", bufs=tmp_bufs) as tpool,
            tc.tile_pool(name="psumT", bufs=pT_bufs, space="PSUM") as ppoolT,
            tc.tile_pool(name="psumS", bufs=pS_bufs, space="PSUM") as ppoolS,
        ):
            ident = cpool.tile([128, 128], XDT)
            nc.sync.dma_start(out=ident[:], in_=ident_d[:])
            if pe_warmup:
                wm_ps = cpool.tile([128, 128], XDT, space="PSUM", name="wm_ps")
                for _ in range(pe_warmup):
                    nc.tensor.transpose(wm_ps[:], ident[:], ident[:])
            pab_sb = cpool.tile([128, 10], BF16)
            nc.sync.dma_start(out=pab_sb[:], in_=pab_d[:])
            coef_sb = cpool.tile([128, 18], F32)
            nc.sync.dma_start(out=coef_sb[:], in_=coef_d[:])

            def poly_and_store(sn_ap, nk, mstart):
                # sn_ap [128, nk*10]; free = 10*K + 2*v + t
                sv = sn_ap.rearrange(
                    "p (K v t) -> p K v t", K=nk, v=5, t=2,
                )
                s = [sv[:, :, v, :] for v in range(5)]
                FREE = nk * 2

                def tmp(tag):
                    tl = tpool.tile([128, FREE], F32, tag=tag)
                    return tl[:].rearrange(
                        "p (K t) -> p K t", K=nk, t=2,
                    )

                # 16-coef multilinear Horner, spread across ACT/DVE/Pool.
                # leaf i: L_i = c_i + c_{8+i} * s0
                Lf = []
                for i in range(8):
                    v = tmp(f"leaf{i}")
                    cb = coef_sb[:, i : i + 1]
                    cs = coef_sb[:, 8 + i : 9 + i]
                    if i < 2:
                        nc.gpsimd.tensor_scalar(
                            v, s[0], cs, cb, op0=ALU.mult, op1=ALU.add
                        )
                    elif i < 4:
                        nc.vector.tensor_scalar(
                            v, s[0], cs, cb, op0=ALU.mult, op1=ALU.add
                        )
                    else:
                        nc.scalar.activation(
                            v, s[0], AF.Identity, bias=cb, scale=cs
                        )
                    Lf.append(v)
                # G_k = L_{2k} + s1 * L_{2k+1}; G0/G2 on DVE, G1/G3 on Pool
                G = []
                for k in range(4):
                    v = tmp(f"gm{k}")
                    eng = nc.vector if k % 2 == 0 else nc.gpsimd
                    eng.tensor_mul(v, s[1], Lf[2 * k + 1])
                    eng.tensor_add(v, v, Lf[2 * k])
                    G.append(v)
                # H_m = G_{2m} + s2 * G_{2m+1}; both on DVE
                H = []
                for m in range(2):
                    v = tmp(f"hm{m}")
                    nc.vector.tensor_mul(v, s[2], G[2 * m + 1])
                    nc.vector.tensor_add(v, v, G[2 * m])
                    H.append(v)
                # add path on s4 = s0+s1+s2 (from PE); Pool
                vr = tmp("relu")
                nc.gpsimd.tensor_scalar(vr, s[4], -2.0, 0.0, op0=ALU.add, op1=ALU.max)
                nc.gpsimd.tensor_scalar(
                    vr, vr, 1.0, coef_sb[:, 16:17], op0=ALU.min, op1=ALU.mult
                )
                vq = tmp("step")
                nc.gpsimd.tensor_scalar(
                    vq, s[4], 2.0, coef_sb[:, 17:18], op0=ALU.is_ge, op1=ALU.mult
                )
                # w = H0 + vr + vq runs parallel to the s3*H1 mul
                w = tmp("wsum")
                nc.gpsimd.tensor_add(w, H[0], vr)
                nc.gpsimd.tensor_add(w, w, vq)
                vl = tmp("lut")
                nc.vector.tensor_mul(vl, s[3], H[1])
                ot = tpool.tile([128, FREE], F32, tag="outsb")
                vo = ot[:].rearrange(
                    "p (K t) -> p K t", K=nk, t=2,
                )
                nc.vector.tensor_add(vo, vl, w)
                nc.sync.dma_start(
                    out=outv[:, mstart : mstart + FREE], in_=ot[:]
                )

            if tail_single and n_chunks >= 4 and BCl == 2:
                batch_of = [2] * (n_chunks // 2 - 1) + [1, 1]
            else:
                batch_of = [BCl] * (n_chunks // BCl)
            chunk_batch = []
            for bi, bs in enumerate(batch_of):
                chunk_batch += [(bi, bs, bj) for bj in range(bs)]
            sn_sb = None
            mdone = 0
            for n in range(n_chunks):
                bi, bs, b = chunk_batch[n]
                xc = xpool.tile([128, TPCl * CIN], XDT, tag="xc")
                if n == 0 or n >= n_chunks - 2:
                    # split first/last loads: shorter pipeline fill/drain
                    Q = TPCl * CIN // 4
                    xvn = xv[n].rearrange("p (q f) -> q p f", q=4)
                    for q in range(4):
                        nc.sync.dma_start(
                            out=xc[:, Q * q : Q * (q + 1)], in_=xvn[q]
                        )
                else:
                    nc.sync.dma_start(out=xc[:], in_=xv[n])
                SNW = SUBl * 10
                if b == 0:
                    sn_sb = snpool.tile([128, bs * SNW], F32, tag="snat")
                halves = 2 if (tail_halves and bs == 1) else 1
                sn_ps_h = []
                for hh in range(halves):
                    sn_ps_t = ppoolS.tile(
                        [128, SNW // halves], F32, tag="sn", name=f"snps_{n}_{hh}"
                    )
                    sn_ps_h.append(sn_ps_t)
                tail = tail_half and n >= n_chunks - 2
                for g in range(SUBl // 4):
                    xt_ps = ppoolT.tile([128, 512], XDT, tag="xt")
                    for j in range(4):
                        k = 4 * g + j
                        nc.tensor.transpose(
                            xt_ps[:, 128 * j : 128 * (j + 1)],
                            xc[:, 128 * k : 128 * (k + 1)],
                            ident[:],
                        )
                    xt_sb = xtpool.tile([128, 512], BF16, tag="xt_sb")
                    if tail:
                        # latency-optimized: both engines copy halves
                        nc.scalar.copy(out=xt_sb[:, :256], in_=xt_ps[:, :256])
                        nc.vector.tensor_copy(
                            out=xt_sb[:, 256:], in_=xt_ps[:, 256:]
                        )
                    elif g % 2 == 0:
                        nc.scalar.copy(out=xt_sb[:], in_=xt_ps[:])
                    else:
                        nc.vector.tensor_copy(out=xt_sb[:], in_=xt_ps[:])
                    KH = SUBl // halves
                    for j in range(4):
                        k = 4 * g + j
                        nc.tensor.matmul(
                            sn_ps_h[k // KH][:, 10 * (k % KH) : 10 * (k % KH + 1)],
                            lhsT=xt_sb[:, 128 * j : 128 * (j + 1)],
                            rhs=pab_sb[:],
                            start=True, stop=True,
                        )
                    if (g + 1) % (SUBl // 4 // halves) == 0:
                        h = (4 * g + 3) // KH
                        HW_ = SNW // halves
                        nc.scalar.copy(
                            out=sn_sb[:, SNW * b + HW_ * h : SNW * b + HW_ * (h + 1)],
                            in_=sn_ps_h[h][:],
                        )
                        if halves == 2:
                            poly_and_store(
                                sn_sb[:, HW_ * h : HW_ * (h + 1)],
                                KH, mdone + KH * 2 * h,
                            )
                if b == bs - 1:
                    if halves == 2:
                        mdone += SUBl * 2
                    else:
                        if poly_deprio:
                            p0 = tc.cur_priority
                            tc.cur_priority = p0 + poly_deprio
                            poly_and_store(
                                sn_sb[:], bs * SUBl, mdone
                            )
                            tc.cur_priority -= poly_deprio
                        else:
                            poly_and_store(sn_sb[:], bs * SUBl, mdone)
                        mdone += bs * SUBl * 2
    nc.compile()
    return nc


def host_prep(choice_parameters, lut, lut_vs_add_choice_parameters):
    cp = np.asarray(choice_parameters, dtype=np.float64)
    e = np.exp(cp - cp.max(axis=0, keepdims=True))
    probs = e / e.sum(axis=0, keepdims=True)  # [64,4]
    L = 1.0 / (1.0 + np.exp(-np.asarray(lut, dtype=np.float64)))
    m = np.asarray(lut_vs_add_choice_parameters, dtype=np.float64)
    em = np.exp(m - m.max())
    mix = em / em.sum()

    c = np.zeros(16)
    for S in range(16):
        v = L
        for ax in range(4):
            vec = np.array([1.0, -1.0]) if (S >> ax) & 1 else np.array([0.0, 1.0])
            v = np.tensordot(v, vec, axes=([0], [0]))
        c[S] = float(v) * mix[0]

    coef_row = np.zeros(18)
    for idx in range(8):
        coef_row[idx] = c[idx << 1]
        coef_row[8 + idx] = c[(idx << 1) | 1]
    coef_row[16] = mix[1]
    coef_row[17] = mix[2]
    coef = np.tile(coef_row.astype(np.float32)[None], (128, 1))

    # pab[q=(t,c), m=(v,t')] = W_v[c] * [t==t']
    # W_v = probs[:, v] for v<4; W_4 = probs[:,0]+probs[:,1]+probs[:,2]
    W = np.zeros((64, 5), np.float64)
    W[:, :4] = probs
    W[:, 4] = probs[:, 0] + probs[:, 1] + probs[:, 2]
    pab = np.zeros((128, 10), np.float64)
    for t in range(2):
        for cc in range(64):
            for v in range(5):
                pab[t * 64 + cc, v * 2 + t] = W[cc, v]
    pab = pab.astype(ml_dtypes.bfloat16)
    return pab, coef


_NC_CACHE = {}


def _get_nc(R):
    if R not in _NC_CACHE:
        _NC_CACHE[R] = build_nc(R)
    return _NC_CACHE[R]


def run_on_hw(x, choice_parameters, lut, lut_vs_add_choice_parameters, **kw):
    x = np.ascontiguousarray(np.asarray(x, dtype=np.float32))
    R = x.shape[0] // N_CORES
    nc = _get_nc(R)
    pab, coef = host_prep(choice_parameters, lut, lut_vs_add_choice_parameters)
    eye = np.eye(128, dtype=np.float32)
    in_maps = [
        {"x": np.ascontiguousarray(x[i * R : (i + 1) * R]), "pab": pab,
         "coef": coef, "ident": eye}
        for i in range(N_CORES)
    ]
    res = run_bass_kernel_spmd(nc, in_maps, list(range(N_CORES)), **kw)
    out = np.concatenate([r["out"] for r in res.results], axis=0)
    return out, res


def kernel(x, choice_parameters, lut, lut_vs_add_choice_parameters):
    out, _ = run_on_hw(x, choice_parameters, lut, lut_vs_add_choice_parameters)
    return out


# revision 25
# speedup vs baseline: 1.7110x; 1.7110x over previous
# Trainium2 Bass kernel for nn_LAB_42906723287350.
#
#   probs = softmax(choice_parameters, axis=0); s = x @ probs
#   out = mix0*multilinear(sigmoid(lut); s) + mix1*clip(s0+s1+s2-2,0,1)
#         + mix2*(s0+s1+s2>=2)
#
# Data parallel over 8 cores (R rows each).  Per core (v3 design):
#   Row space is partition-blocked: partition p owns rows
#   [p*R/128, (p+1)*R/128).  x loaded in 16 chunks [128, 4096] f32
#   (16 KB/partition DMA lines; chunk n covers rows RP*p + 64n + t).
#   Each chunk: 32 PE transposes [128,128] -> PSUM (4/bank) -> ACT/DVE
#   copy to SBUF (bf16 cast) -> 32 tiny matmuls with the x-transpose as
#   STATIONARY and a constant [128,10] bf16 "pab" as MOVING operand:
#   psum[p, 10k+2v+t] = s_v(row) for v in {s0..s3, s0+s1+s2} -- s lands
#   in natural layout, no back-transposes.  One PSUM bank per chunk,
#   one ACT copy -> SBUF.  Poly (16-leaf Horner + add path) runs per
#   2-chunk batch spread across ACT/DVE/Pool; each batch stores with a
#   single full-rate DMA (512 B runs).
import numpy as np
import ml_dtypes

import concourse.bass as bass
import concourse.mybir as mybir
import concourse.tile as tile
from concourse import bacc
from concourse.bass_utils import run_bass_kernel_spmd

N_CORES = 8
B_FULL = 1048576
CIN = 64
F32 = mybir.dt.float32
F32R = mybir.dt.float32r
BF16 = mybir.dt.bfloat16
TPC = 64          # rows per partition per chunk
CHUNK_ROWS = 128 * TPC          # 8192
SUB_PER_CHUNK = TPC // 2        # 32 transpose sub-tiles per chunk
BATCH_CHUNKS = 2                # poly batch = 2 chunks = 16384 rows
ALU = mybir.AluOpType
AF = mybir.ActivationFunctionType


def build_nc(R, tpc=TPC, batch_chunks=BATCH_CHUNKS, xin_bufs=4, xt_bufs=6,
             sn_bufs=3, tmp_bufs=2, pT_bufs=4, pS_bufs=3, split_first=4,
             tail_single=True, tail_half=False, poly_deprio=250,
             x_f32r=True, tail_halves=False, pe_warmup=0, x_bf16=True):
    XDT = F32R if x_f32r else F32
    if x_bf16:
        XDT = BF16
    TPCl = tpc
    CHUNK_ROWSl = 128 * TPCl
    SUBl = TPCl // 2
    BCl = batch_chunks
    assert R % (BCl * CHUNK_ROWSl) == 0
    n_chunks = R // CHUNK_ROWSl
    n_batches = n_chunks // BCl
    MB = BCl * TPCl     # out rows per partition per batch

    nc = bacc.Bacc()
    x_d = nc.dram_tensor("x", [R, CIN], F32 if x_bf16 else XDT,
                         kind="ExternalInput")
    pab_d = nc.dram_tensor("pab", [128, 10], BF16, kind="ExternalInput")
    coef_d = nc.dram_tensor("coef", [128, 18], F32, kind="ExternalInput")
    out_d = nc.dram_tensor("out", [R, 1], F32, kind="ExternalOutput")
    ident_d = nc.dram_tensor("ident", [128, 128], XDT, kind="ExternalInput")

    # x row = (R/128)*p + 64*n + t  (16 KB contiguous per partition line)
    xv = x_d[:].rearrange("(p n t) c -> n p (t c)", p=128, n=n_chunks, t=TPCl)
    # out row = (R/128)*p + m ; one full-rate store per poly batch
    outv = out_d[:].rearrange("(p m) one -> p (m one)", p=128)

    with tile.TileContext(nc) as tc:
        with (
            tc.tile_pool(name="const", bufs=1) as cpool,
            tc.tile_pool(name="xin", bufs=xin_bufs) as xpool,
            tc.tile_pool(name="xtsb", bufs=xt_bufs) as xtpool,
            tc.tile_pool(name="snat", bufs=sn_bufs) as snpool,
            tc.tile_pool(name="tmp", bufs=tmp_bufs) as tpool,
            tc.tile_pool(name="psumT", bufs=pT_bufs, space="PSUM") as ppoolT,
            tc.tile_pool(name="psumS", bufs=pS_bufs, space="PSUM") as ppoolS,
        ):
            ident = cpool.tile([128, 128], XDT)
            nc.sync.dma_start(out=ident[:], in_=ident_d[:])
            if pe_warmup:
                wm_ps = cpool.tile([128, 128], XDT, space="PSUM", name="wm_ps")
                for _ in range(pe_warmup):
                    nc.tensor.transpose(wm_ps[:], ident[:], ident[:])
            pab_sb = cpool.tile([128, 10], BF16)
            nc.sync.dma_start(out=pab_sb[:], in_=pab_d[:])
            coef_sb = cpool.tile([128, 18], F32)
            nc.sync.dma_start(out=coef_sb[:], in_=coef_d[:])

            def poly_and_store(sn_ap, nk, mstart):
                # sn_ap [128, nk*10]; free = 10*K + 2*v + t
                sv = sn_ap.rearrange(
                    "p (K v t) -> p K v t", K=nk, v=5, t=2,
                )
                s = [sv[:, :, v, :] for v in range(5)]
                FREE = nk * 2

                def tmp(tag):
                    tl = tpool.tile([128, FREE], F32, tag=tag)
                    return tl[:].rearrange(
                        "p (K t) -> p K t", K=nk, t=2,
                    )

                # 16-coef multilinear Horner, spread across ACT/DVE/Pool.
                # leaf i: L_i = c_i + c_{8+i} * s0
                Lf = []
                for i in range(8):
                    v = tmp(f"leaf{i}")
                    cb = coef_sb[:, i : i + 1]
                    cs = coef_sb[:, 8 + i : 9 + i]
                    if i < 2:
                        nc.gpsimd.tensor_scalar(
                            v, s[0], cs, cb, op0=ALU.mult, op1=ALU.add
                        )
                    elif i < 4:
                        nc.vector.tensor_scalar(
                            v, s[0], cs, cb, op0=ALU.mult, op1=ALU.add
                        )
                    else:
                        nc.scalar.activation(
                            v, s[0], AF.Identity, bias=cb, scale=cs
                        )
                    Lf.append(v)
                # G_k = L_{2k} + s1 * L_{2k+1}; G0/G2 on DVE, G1/G3 on Pool
                G = []
                for k in range(4):
                    v = tmp(f"gm{k}")
                    eng = nc.vector if k % 2 == 0 else nc.gpsimd
                    eng.tensor_mul(v, s[1], Lf[2 * k + 1])
                    eng.tensor_add(v, v, Lf[2 * k])
                    G.append(v)
                # H_m = G_{2m} + s2 * G_{2m+1}; both on DVE
                H = []
                for m in range(2):
                    v = tmp(f"hm{m}")
                    nc.vector.tensor_mul(v, s[2], G[2 * m + 1])
                    nc.vector.tensor_add(v, v, G[2 * m])
                    H.append(v)
                # add path on s4 = s0+s1+s2 (from PE); Pool
                vr = tmp("relu")
                nc.gpsimd.tensor_scalar(vr, s[4], -2.0, 0.0, op0=ALU.add, op1=ALU.max)
                nc.gpsimd.tensor_scalar(
                    vr, vr, 1.0, coef_sb[:, 16:17], op0=ALU.min, op1=ALU.mult
                )
                vq = tmp("step")
                nc.gpsimd.tensor_scalar(
                    vq, s[4], 2.0, coef_sb[:, 17:18], op0=ALU.is_ge, op1=ALU.mult
                )
                # w = H0 + vr + vq runs parallel to the s3*H1 mul
                w = tmp("wsum")
                nc.gpsimd.tensor_add(w, H[0], vr)
                nc.gpsimd.tensor_add(w, w, vq)
                vl = tmp("lut")
                nc.vector.tensor_mul(vl, s[3], H[1])
                ot = tpool.tile([128, FREE], F32, tag="outsb")
                vo = ot[:].rearrange(
                    "p (K t) -> p K t", K=nk, t=2,
                )
                nc.vector.tensor_add(vo, vl, w)
                nc.sync.dma_start(
                    out=outv[:, mstart : mstart + FREE], in_=ot[:]
                )

            if tail_single and n_chunks >= 4 and BCl == 2:
                batch_of = [2] * (n_chunks // 2 - 1) + [1, 1]
            else:
                batch_of = [BCl] * (n_chunks // BCl)
            chunk_batch = []
            for bi, bs in enumerate(batch_of):
                chunk_batch += [(bi, bs, bj) for bj in range(bs)]
            sn_sb = None
            mdone = 0
            for n in range(n_chunks):
                bi, bs, b = chunk_batch[n]
                xc = xpool.tile([128, TPCl * CIN], XDT, tag="xc")
                dma_eng = nc.gpsimd if x_bf16 else nc.sync
                if n == 0 or n >= n_chunks - 2:
                    # split first/last loads: shorter pipeline fill/drain
                    Q = TPCl * CIN // 4
                    xvn = xv[n].rearrange("p (q f) -> q p f", q=4)
                    for q in range(4):
                        dma_eng.dma_start(
                            out=xc[:, Q * q : Q * (q + 1)], in_=xvn[q]
                        )
                else:
                    dma_eng.dma_start(out=xc[:], in_=xv[n])
                SNW = SUBl * 10
                if b == 0:
                    sn_sb = snpool.tile([128, bs * SNW], F32, tag="snat")
                halves = 2 if (tail_halves and bs == 1) else 1
                sn_ps_h = []
                for hh in range(halves):
                    sn_ps_t = ppoolS.tile(
                        [128, SNW // halves], F32, tag="sn", name=f"snps_{n}_{hh}"
                    )
                    sn_ps_h.append(sn_ps_t)
                tail = tail_half and n >= n_chunks - 2
                GS = 8 if x_bf16 else 4      # sub-tiles per PSUM bank
                for g in range(SUBl // GS):
                    xt_ps = ppoolT.tile([128, 128 * GS], XDT, tag="xt")
                    for j in range(GS):
                        k = GS * g + j
                        nc.tensor.transpose(
                            xt_ps[:, 128 * j : 128 * (j + 1)],
                            xc[:, 128 * k : 128 * (k + 1)],
                            ident[:],
                        )
                    xt_sb = xtpool.tile([128, 128 * GS], BF16, tag="xt_sb")
                    HALF = 64 * GS
                    if tail:
                        # latency-optimized: both engines copy halves
                        nc.scalar.copy(out=xt_sb[:, :HALF], in_=xt_ps[:, :HALF])
                        nc.vector.tensor_copy(
                            out=xt_sb[:, HALF:], in_=xt_ps[:, HALF:]
                        )
                    elif g % 2 == 0:
                        nc.scalar.copy(out=xt_sb[:], in_=xt_ps[:])
                    else:
                        nc.vector.tensor_copy(out=xt_sb[:], in_=xt_ps[:])
                    KH = SUBl // halves
                    for j in range(GS):
                        k = GS * g + j
                        nc.tensor.matmul(
                            sn_ps_h[k // KH][:, 10 * (k % KH) : 10 * (k % KH + 1)],
                            lhsT=xt_sb[:, 128 * j : 128 * (j + 1)],
                            rhs=pab_sb[:],
                            start=True, stop=True,
                        )
                    if (g + 1) % (SUBl // GS // halves) == 0:
                        h = (GS * g + GS - 1) // KH
                        HW_ = SNW // halves
                        nc.scalar.copy(
                            out=sn_sb[:, SNW * b + HW_ * h : SNW * b + HW_ * (h + 1)],
                            in_=sn_ps_h[h][:],
                        )
                        if halves == 2:
                            poly_and_store(
                                sn_sb[:, HW_ * h : HW_ * (h + 1)],
                                KH, mdone + KH * 2 * h,
                            )
                if b == bs - 1:
                    if halves == 2:
                        mdone += SUBl * 2
                    else:
                        if poly_deprio:
                            p0 = tc.cur_priority
                            tc.cur_priority = p0 + poly_deprio
                            poly_and_store(
                                sn_sb[:], bs * SUBl, mdone
                            )
                            tc.cur_priority -= poly_deprio
                        else:
                            poly_and_store(sn_sb[:], bs * SUBl, mdone)
                        mdone += bs * SUBl * 2
    nc.compile()
    return nc


def host_prep(choice_parameters, lut, lut_vs_add_choice_parameters):
    cp = np.asarray(choice_parameters, dtype=np.float64)
    e = np.exp(cp - cp.max(axis=0, keepdims=True))
    probs = e / e.sum(axis=0, keepdims=True)  # [64,4]
    L = 1.0 / (1.0 + np.exp(-np.asarray(lut, dtype=np.float64)))
    m = np.asarray(lut_vs_add_choice_parameters, dtype=np.float64)
    em = np.exp(m - m.max())
    mix = em / em.sum()

    c = np.zeros(16)
    for S in range(16):
        v = L
        for ax in range(4):
            vec = np.array([1.0, -1.0]) if (S >> ax) & 1 else np.array([0.0, 1.0])
            v = np.tensordot(v, vec, axes=([0], [0]))
        c[S] = float(v) * mix[0]

    coef_row = np.zeros(18)
    for idx in range(8):
        coef_row[idx] = c[idx << 1]
        coef_row[8 + idx] = c[(idx << 1) | 1]
    coef_row[16] = mix[1]
    coef_row[17] = mix[2]
    coef = np.tile(coef_row.astype(np.float32)[None], (128, 1))

    # pab[q=(t,c), m=(v,t')] = W_v[c] * [t==t']
    # W_v = probs[:, v] for v<4; W_4 = probs[:,0]+probs[:,1]+probs[:,2]
    W = np.zeros((64, 5), np.float64)
    W[:, :4] = probs
    W[:, 4] = probs[:, 0] + probs[:, 1] + probs[:, 2]
    pab = np.zeros((128, 10), np.float64)
    for t in range(2):
        for cc in range(64):
            for v in range(5):
                pab[t * 64 + cc, v * 2 + t] = W[cc, v]
    pab = pab.astype(ml_dtypes.bfloat16)
    return pab, coef


_NC_CACHE = {}


def _get_nc(R):
    if R not in _NC_CACHE:
        _NC_CACHE[R] = build_nc(R)
    return _NC_CACHE[R]


def run_on_hw(x, choice_parameters, lut, lut_vs_add_choice_parameters, **kw):
    x = np.ascontiguousarray(np.asarray(x, dtype=np.float32))
    R = x.shape[0] // N_CORES
    nc = _get_nc(R)
    pab, coef = host_prep(choice_parameters, lut, lut_vs_add_choice_parameters)
    ident_dt = np.float32
    for a in nc.m.functions[0].allocations:
        if isinstance(a, mybir.MemoryLocationSet) and a.memorylocations and \
                a.memorylocations[0].name == "ident":
            ident_dt = mybir.dt.np(a.dtype)
    eye = np.eye(128, dtype=np.float32).astype(ident_dt)
    in_maps = [
        {"x": np.ascontiguousarray(x[i * R : (i + 1) * R]), "pab": pab,
         "coef": coef, "ident": eye}
        for i in range(N_CORES)
    ]
    res = run_bass_kernel_spmd(nc, in_maps, list(range(N_CORES)), **kw)
    out = np.concatenate([r["out"] for r in res.results], axis=0)
    return out, res


def kernel(x, choice_parameters, lut, lut_vs_add_choice_parameters):
    out, _ = run_on_hw(x, choice_parameters, lut, lut_vs_add_choice_parameters)
    return out
